# revision 14
# baseline (speedup 1.0000x reference)
"""Trainium2 Bass kernel for nn_CentersDistance.

logits[k, n] = -||centers[k] - inputs[n]||^2
             = 2*(centers @ inputs.T)[k, n] - ||centers[k]||^2 - ||inputs[n]||^2

Strategy (8 NeuronCores, data-parallel over N):
  * host: transpose both operands so the contraction dim D lands on the SBUF
    partition axis ([D, K] and [D, N] layouts), fold the factor 2 into the
    inputs, precompute the (exact, float64) norm terms.
  * each core: 1024x1024x1024 matmul in bf16 (fp32 PSUM accumulation),
    epilogue on DVE adds -||c||^2 (per-partition scalar) and -||x||^2
    (broadcast row) in a single scalar_tensor_tensor op, store fp32.
"""

import threading
from contextlib import ExitStack

import numpy as np
import ml_dtypes

import concourse.bass as bass
import concourse.mybir as mybir
import concourse.tile as tile
from concourse import bacc
from concourse.bass_utils import run_bass_kernel_spmd

N_CORES = 8
N, K, D = 8192, 1024, 1024
NSH = N // N_CORES  # per-core slab of inputs
P = 128             # SBUF partitions
NF = 512            # matmul moving free dim (one fp32 PSUM bank)

D_TILES = D // P    # 8
M_TILES = K // P    # 8
H_TILES = NSH // NF # 2

_DT = mybir.dt.bfloat16
_NP_DT = ml_dtypes.bfloat16

_cache = threading.local()


def _build_nc():
    nc = bacc.Bacc(
        "TRN2", target_bir_lowering=False, debug=False, num_devices=N_CORES
    )
    ct = nc.dram_tensor("ct", [D, K], _DT, kind="ExternalInput").ap()
    xt = nc.dram_tensor("xt", [D, NSH], _DT, kind="ExternalInput").ap()
    ncsq = nc.dram_tensor(
        "ncsq", [P, M_TILES], mybir.dt.float32, kind="ExternalInput"
    ).ap()
    nxsq = nc.dram_tensor(
        "nxsq", [P, NSH], mybir.dt.float32, kind="ExternalInput"
    ).ap()
    out = nc.dram_tensor("out", [K, NSH], mybir.dt.float32, kind="ExternalOutput").ap()

    ct_r = ct.rearrange("(t p) k -> t p k", p=P)
    xt_r = xt.rearrange("(t p) n -> t p n", p=P)
    out_r = out.rearrange("(m p) n -> m p n", p=P)

    with tile.TileContext(nc) as tc:
        with (
            tc.tile_pool(name="w", bufs=1) as wpool,
            tc.tile_pool(name="c", bufs=1) as cpool,
            tc.tile_pool(name="o", bufs=4) as opool,
            tc.tile_pool(name="ps", bufs=7, space="PSUM") as pspool,
            tc.tile_pool(name="wu", bufs=1, space="PSUM") as wupool,
        ):
            # PE warm-up: ~dummy matmuls on a zeroed tile, no data deps, so
            # the tensor engine is busy during the load phase and the HAM
            # clock gate is fully open (2.4 GHz) when the real matmuls start.
            wu_sb = cpool.tile([P, NF], _DT, tag="wu_sb")
            nc.gpsimd.memset(wu_sb[:], 0.0)
            wu_ps = wupool.tile([P, NF], mybir.dt.float32, tag="wu_ps")
            for _ in range(4):
                nc.tensor.matmul(
                    wu_ps[:], wu_sb[:, 0:P], wu_sb[:], start=True, stop=True
                )

            ct_sb = []
            xt_sb = []
            for d in range(D_TILES):
                t = wpool.tile([P, K], _DT, tag=f"ct{d}")
                nc.sync.dma_start(t[:], ct_r[d])
                ct_sb.append(t)
                t = wpool.tile([P, NSH], _DT, tag=f"xt{d}")
                nc.sync.dma_start(t[:], xt_r[d])
                xt_sb.append(t)
                if d == 3:
                    # epilogue constants — needed much later than the ct/xt
                    # tiles; use the GpSimd DMA queue so they don't serialize
                    # with the load stream on the Sync queue
                    ncsq_sb = cpool.tile([P, M_TILES], mybir.dt.float32, tag="ncsq")
                    nc.gpsimd.dma_start(ncsq_sb[:], ncsq)
                    nxsq_sb = cpool.tile([P, NSH], mybir.dt.float32, tag="nxsq")
                    nc.gpsimd.dma_start(nxsq_sb[:], nxsq)

            def epilogue(m, h, ps):
                ot = opool.tile([P, NF], mybir.dt.float32, tag="ot", name=f"ot{m}_{h}")
                nc.vector.scalar_tensor_tensor(
                    ot[:],
                    ps[:],
                    ncsq_sb[:, m : m + 1],
                    nxsq_sb[:, h * NF : (h + 1) * NF],
                    op0=mybir.AluOpType.add,
                    op1=mybir.AluOpType.add,
                )
                nc.sync.dma_start(out_r[m][:, h * NF : (h + 1) * NF], ot[:])

            # Pass 1 (m-tiles 0-3): d outermost so the matmuls pace with the
            # streaming ct/xt DMAs; 8 PSUM banks accumulate concurrently.
            ms = range(M_TILES // 2)
            ps = {}
            for m in ms:
                for h in range(H_TILES):
                    ps[(m, h)] = pspool.tile(
                        [P, NF], mybir.dt.float32, tag="ps", name=f"ps_{m}_{h}"
                    )
            for d in range(D_TILES):
                for m in ms:
                    for h in range(H_TILES):
                        nc.tensor.matmul(
                            ps[(m, h)][:],
                            ct_sb[d][:, m * P : (m + 1) * P],
                            xt_sb[d][:, h * NF : (h + 1) * NF],
                            start=(d == 0),
                            stop=(d == D_TILES - 1),
                        )
            for m in ms:
                for h in range(H_TILES):
                    epilogue(m, h, ps[(m, h)])

            # Pass 2 (m-tiles 4-7): everything is resident now, so run d
            # innermost — each (m, h) output retires early and its DVE
            # epilogue + store overlap the remaining matmuls instead of
            # serializing at the kernel tail.
            for m in range(M_TILES // 2, M_TILES):
                for h in range(H_TILES):
                    p2 = pspool.tile(
                        [P, NF], mybir.dt.float32, tag="ps", name=f"ps2_{m}_{h}"
                    )
                    for d in range(D_TILES):
                        nc.tensor.matmul(
                            p2[:],
                            ct_sb[d][:, m * P : (m + 1) * P],
                            xt_sb[d][:, h * NF : (h + 1) * NF],
                            start=(d == 0),
                            stop=(d == D_TILES - 1),
                        )
                    epilogue(m, h, p2)

    nc.compile()
    return nc


def _build_nc_raw():
    """Raw-Block implementation: same dataflow as the Tile version but with
    5 hand-placed semaphores, because the NEFF epilogue resets every
    declared semaphore serially (~115 ns each) — Tile's ~50 sems cost ~6 us
    of pure tail on every run."""
    nc = bacc.Bacc(
        "TRN2", target_bir_lowering=False, debug=False, num_devices=N_CORES
    )
    ct = nc.dram_tensor("ct", [D, K], _DT, kind="ExternalInput").ap()
    xt = nc.dram_tensor("xt", [D, NSH], _DT, kind="ExternalInput").ap()
    ncsq = nc.dram_tensor(
        "ncsq", [P, M_TILES], mybir.dt.float32, kind="ExternalInput"
    ).ap()
    nxsq = nc.dram_tensor(
        "nxsq", [P, NSH], mybir.dt.float32, kind="ExternalInput"
    ).ap()
    out = nc.dram_tensor("out", [K, NSH], mybir.dt.float32, kind="ExternalOutput").ap()

    ct_r = ct.rearrange("(t p) k -> t p k", p=P)
    xt_r = xt.rearrange("(t p) n -> t p n", p=P)
    out_r = out.rearrange("(m p) n -> m p n", p=P)

    G = M_TILES * H_TILES          # 16 output groups of [128, 512]
    GP1 = G // 2                   # groups 0-7 -> pass 1 (m 0-3)
    N_WU = 6                       # PE warm-up matmuls

    def g_mh(g):
        return g // H_TILES, g % H_TILES

    with (
        nc.sbuf_tensor("wu_sb", [P, NF], _DT) as wu_sb,
        nc.sbuf_tensor("ncsq_sb", [P, M_TILES], mybir.dt.float32) as ncsq_sb,
        nc.sbuf_tensor("nxsq_sb", [P, NSH], mybir.dt.float32) as nxsq_sb,
        nc.sbuf_tensor("ot_sb", [P, G * NF], mybir.dt.float32) as ot_sb,
    ):
        with (
            ExitStack() as stack,
            nc.semaphore() as ctb_sem,
            nc.semaphore() as const_sem,
            nc.semaphore() as mm_sem,
            nc.semaphore() as dve_sem,
            nc.semaphore() as dma_out,
            nc.Block() as block,
        ):
            # one sem per d-tile pair: HW-DGE completions of equal-size DMAs
            # are *usually* in issue order, but HBM contention from the other
            # 7 cores can invert them — a shared counter would then let the
            # PE read a tile that is not fully written.
            d_sems = [
                stack.enter_context(nc.semaphore(f"d_sem{i}"))
                for i in range(D_TILES)
            ]
            ct_sb = [
                stack.enter_context(nc.sbuf_tensor(f"ct_sb{d}", [P, K], _DT))
                for d in range(D_TILES)
            ]
            xt_sb = [
                stack.enter_context(nc.sbuf_tensor(f"xt_sb{d}", [P, NSH], _DT))
                for d in range(D_TILES)
            ]
            ps = [
                stack.enter_context(
                    nc.psum_tensor(f"ps{b}", [P, NF], mybir.dt.float32)
                )
                for b in range(GP1)
            ]

            @block.sync
            def _(sync):
                # xt on the Sync HW-DGE queue; ct goes out in parallel on the
                # Scalar engine's queue (see block.scalar below) — two rings
                # halve the time to the first d-tile pair and keep the d-loop
                # ahead of the PE throughout
                for d in range(D_TILES):
                    sync.dma_start(xt_sb[d][:], xt_r[d]).then_inc(d_sems[d], 16)
                # consts last: only the DVE epilogue needs them
                sync.dma_start(ncsq_sb[:], ncsq).then_inc(const_sem, 16)
                sync.dma_start(nxsq_sb[:], nxsq).then_inc(const_sem, 16)
                for g in range(G):
                    m, h = g_mh(g)
                    sync.wait_ge(dve_sem, g + 1)
                    sync.dma_start(
                        out_r[m][:, h * NF : (h + 1) * NF],
                        ot_sb[:, g * NF : (g + 1) * NF],
                    ).then_inc(dma_out, 16)
                sync.wait_ge(dma_out, G * 16)

            @block.scalar
            def _(scalar):
                for d in range(D_TILES):
                    scalar.dma_start(ct_sb[d][:], ct_r[d]).then_inc(d_sems[d], 16)

            @block.tensor
            def _(tensor):
                # warm-up: open the HAM clock gate while the loads stream.
                # wu_sb is deliberately uninitialized — the products are never
                # read, only the PE-busy time matters.
                for _ in range(N_WU):
                    nc.tensor.matmul(
                        ps[GP1 - 1][:], wu_sb[:, 0:P], wu_sb[:], start=True, stop=True
                    )
                # pass 1: groups 0-7 accumulate in banks 0-7, d outermost so
                # matmuls pace with the streaming loads
                for d in range(D_TILES):
                    tensor.wait_ge(d_sems[d], 32)
                    for g in range(GP1):
                        m, h = g_mh(g)
                        mm = nc.tensor.matmul(
                            ps[g][:],
                            ct_sb[d][:, m * P : (m + 1) * P],
                            xt_sb[d][:, h * NF : (h + 1) * NF],
                            start=(d == 0),
                            stop=(d == D_TILES - 1),
                        )
                        if d == D_TILES - 1:
                            mm.then_inc(mm_sem, 1)
                # pass 2: groups 8-15 reuse banks 0-7 once the DVE epilogue
                # has drained the pass-1 group from that bank
                for g in range(GP1, G):
                    m, h = g_mh(g)
                    tensor.wait_ge(dve_sem, g - GP1 + 1)
                    for d in range(D_TILES):
                        mm = nc.tensor.matmul(
                            ps[g - GP1][:],
                            ct_sb[d][:, m * P : (m + 1) * P],
                            xt_sb[d][:, h * NF : (h + 1) * NF],
                            start=(d == 0),
                            stop=(d == D_TILES - 1),
                        )
                    mm.then_inc(mm_sem, 1)

            @block.vector
            def _(vector):
                vector.wait_ge(const_sem, 32)  # consts present
                for g in range(G):
                    m, h = g_mh(g)
                    vector.wait_ge(mm_sem, g + 1)
                    nc.vector.scalar_tensor_tensor(
                        ot_sb[:, g * NF : (g + 1) * NF],
                        ps[g % GP1][:],
                        ncsq_sb[:, m : m + 1],
                        nxsq_sb[:, h * NF : (h + 1) * NF],
                        op0=mybir.AluOpType.add,
                        op1=mybir.AluOpType.add,
                    ).then_inc(dve_sem, 1)

    nc.compile()
    return nc


def _get_nc():
    if not hasattr(_cache, "nc"):
        _cache.nc = _build_nc_raw()
    return _cache.nc


def kernel(inputs, centers, _trace=False):
    inputs = np.asarray(inputs, dtype=np.float32)
    centers = np.asarray(centers, dtype=np.float32)

    csq = np.sum(centers.astype(np.float64) ** 2, axis=1)
    xsq = np.sum(inputs.astype(np.float64) ** 2, axis=1)

    ct = np.ascontiguousarray(centers.T).astype(_NP_DT)
    xt2 = np.ascontiguousarray((2.0 * inputs).T.astype(_NP_DT))
    ncsq = np.ascontiguousarray(
        (-csq).reshape(M_TILES, P).T.astype(np.float32)
    )

    in_maps = []
    for i in range(N_CORES):
        sl = slice(i * NSH, (i + 1) * NSH)
        in_maps.append(
            {
                "ct": ct,
                "xt": np.ascontiguousarray(xt2[:, sl]),
                "ncsq": ncsq,
                "nxsq": np.ascontiguousarray(
                    np.broadcast_to(-xsq[sl], (P, NSH))
                ).astype(np.float32),
            }
        )

    nc = _get_nc()
    res = run_bass_kernel_spmd(
        nc, in_maps, core_ids=list(range(N_CORES)), trace=_trace
    )
    if _trace:
        kernel.last_results = res
    out = np.concatenate([r["out"] for r in res.results], axis=1)
    return out


# revision 18
# speedup vs baseline: 1.0345x; 1.0345x over previous
"""Trainium2 Bass kernel for nn_CentersDistance (retrieval_knn).

logits[k, n] = -||centers[k] - inputs[n]||^2
             = 2*(centers @ inputs.T)[k, n] - ||centers[k]||^2 - ||inputs[n]||^2

Strategy (8 NeuronCores, data-parallel over the N=8192 inputs):
  * host: transpose both operands so the contraction dim D lands on the SBUF
    partition axis ([D, K] and [D, N/8] layouts), fold the factor 2 into the
    inputs, and precompute the norm terms exactly in float64.
  * device (per core): a 1024x1024x1024 matmul in bf16 with fp32 PSUM
    accumulation (bf16 streams 1 row/cycle on the PE vs 4 for fp32; the
    measured end-to-end error is absmax/scale 3.3e-4, resid_var 5.4e-9,
    because the exact norm terms dominate the logits).  The epilogue runs on
    the DVE: one scalar_tensor_tensor op adds -||c||^2 (per-partition scalar)
    and -||x||^2 (broadcast row read from a host-precomputed [128, N/8]
    tile), output stored fp32.
  * raw Block/semaphore implementation (not Tile): Tile's ~50 semaphores are
    not the issue (the NRT pre/postamble resets a fixed 51 per engine), but
    Tile adds its own ~6 us drain + clear-semaphores + barrier tail, and its
    scheduler cannot express the exact warmup/pacing we want.
  * the PE is kept continuously busy from ~1 us into the kernel by N_WU
    throwaway matmuls on an (uninitialized) scratch tile so the HAM clock
    gate is fully open (2.4 GHz) when the first real matmul issues; the
    warmup count is sized to bridge until the first ct/xt tile pair lands.
  * loads stream on two HW-DGE queues (Sync: xt, Scalar: ct) with one
    semaphore per d-tile pair: completions of equal-size DMAs are usually in
    issue order, but HBM contention from the other 7 cores can invert them,
    and a single shared counter would then let the PE read a tile that is
    not fully written (observed as a sporadic inf in the output).
  * pass 1 (m-tiles 0-3) runs d outermost so matmuls pace with the streaming
    loads across 8 concurrent PSUM banks; pass 2 (m-tiles 4-7) runs d
    innermost so each output group retires early and its epilogue + store
    overlap the remaining matmuls.

Measured on 8 axon-tunneled trn2 cores: ~45 us NEFF exec (NTFF), of which
~27.6 us is the bf16 PE-stream floor (128 matmuls x 512 rows @ 2.4 GHz) and
~14 us is fixed NRT preamble/postamble (sync barriers, 51-semaphore reset
chains, dma_rearm).

A float32r variant (dt=mybir.dt.float32r, np_dt=np.float32) measures
~56 us / absmax 2.0e-5 — load-bound (8.5 MB vs 4.5 MB of input) but with
near-fp32 precision; kept as a fallback should tighter accuracy ever be
needed.  An fp8e4m3 DoubleRow variant measured ~36 us / absmax 5.2e-3 —
rejected for accuracy-risk reasons.
"""

import threading
from contextlib import ExitStack

import numpy as np
import ml_dtypes

import concourse.mybir as mybir
from concourse import bacc
from concourse.bass_utils import run_bass_kernel_spmd

N_CORES = 8
N, K, D = 8192, 1024, 1024
NSH = N // N_CORES  # per-core slab of inputs
P = 128             # SBUF partitions
NF = 512            # matmul moving free dim (one fp32 PSUM bank)

D_TILES = D // P    # 8 contraction tiles
M_TILES = K // P    # 8 center tiles
H_TILES = NSH // NF # 2 moving-dim tiles

G = M_TILES * H_TILES  # 16 output groups of [128, 512]
GP1 = G // 2           # groups 0-7 -> pass 1 (m-tiles 0-3)
N_WU = 10              # PE warm-up matmuls

_DT = mybir.dt.bfloat16
_NP_DT = ml_dtypes.bfloat16

_cache = threading.local()


def _g_mh(g):
    return g // H_TILES, g % H_TILES


def _build_nc(dt=_DT):
    nc = bacc.Bacc(
        "TRN2", target_bir_lowering=False, debug=False, num_devices=N_CORES
    )
    ct = nc.dram_tensor("ct", [D, K], dt, kind="ExternalInput").ap()
    xt = nc.dram_tensor("xt", [D, NSH], dt, kind="ExternalInput").ap()
    ncsq = nc.dram_tensor(
        "ncsq", [P, M_TILES], mybir.dt.float32, kind="ExternalInput"
    ).ap()
    nxsq = nc.dram_tensor(
        "nxsq", [P, NSH], mybir.dt.float32, kind="ExternalInput"
    ).ap()
    out = nc.dram_tensor("out", [K, NSH], mybir.dt.float32, kind="ExternalOutput").ap()

    ct_r = ct.rearrange("(t p) k -> t p k", p=P)
    xt_r = xt.rearrange("(t p) n -> t p n", p=P)
    out_r = out.rearrange("(m p) n -> m p n", p=P)

    HNF = NF // 2

    with (
        nc.sbuf_tensor("wu_sb", [P, NF], dt) as wu_sb,
        nc.sbuf_tensor("ncsq_sb", [P, M_TILES], mybir.dt.float32) as ncsq_sb,
        nc.sbuf_tensor("nxsq_sb", [P, NSH], mybir.dt.float32) as nxsq_sb,
        nc.sbuf_tensor("ot_sb", [P, G * NF], mybir.dt.float32) as ot_sb,
        ExitStack() as stack,
        nc.semaphore("const_sem") as const_sem,
        nc.semaphore("mm_sem") as mm_sem,
        nc.semaphore("dve_sem") as dve_sem,
        nc.semaphore("dma_out") as dma_out,
        nc.Block() as block,
    ):
        d_sems = [
            stack.enter_context(nc.semaphore(f"d_sem{i}")) for i in range(D_TILES)
        ]
        ct_sb = [
            stack.enter_context(nc.sbuf_tensor(f"ct_sb{d}", [P, K], dt))
            for d in range(D_TILES)
        ]
        xt_sb = [
            stack.enter_context(nc.sbuf_tensor(f"xt_sb{d}", [P, NSH], dt))
            for d in range(D_TILES)
        ]
        ps = [
            stack.enter_context(nc.psum_tensor(f"ps{b}", [P, NF], mybir.dt.float32))
            for b in range(GP1)
        ]

        @block.sync
        def _(sync):
            # xt on the Sync HW-DGE queue; ct goes out in parallel on the
            # Scalar engine's queue (block.scalar below) — two rings halve
            # the time to the first d-tile pair and keep the d-loop ahead
            # of the PE throughout
            for d in range(D_TILES):
                sync.dma_start(xt_sb[d][:], xt_r[d]).then_inc(d_sems[d], 16)
            # consts last: only the DVE epilogue (which runs late) needs them
            sync.dma_start(ncsq_sb[:], ncsq).then_inc(const_sem, 16)
            sync.dma_start(nxsq_sb[:], nxsq).then_inc(const_sem, 16)
            for g in range(G - 1):
                m, h = _g_mh(g)
                sync.wait_ge(dve_sem, g + 1)
                sync.dma_start(
                    out_r[m][:, h * NF : (h + 1) * NF],
                    ot_sb[:, g * NF : (g + 1) * NF],
                ).then_inc(dma_out, 16)
            # last group is split in half so its store starts while the DVE
            # is still draining the second half — shorter kernel tail
            m, h = _g_mh(G - 1)
            for half in range(2):
                sync.wait_ge(dve_sem, G + half)
                sync.dma_start(
                    out_r[m][:, h * NF + half * HNF : h * NF + (half + 1) * HNF],
                    ot_sb[
                        :,
                        (G - 1) * NF + half * HNF : (G - 1) * NF + (half + 1) * HNF,
                    ],
                ).then_inc(dma_out, 16)
            sync.wait_ge(dma_out, (G + 1) * 16)

        @block.scalar
        def _(scalar):
            for d in range(D_TILES):
                scalar.dma_start(ct_sb[d][:], ct_r[d]).then_inc(d_sems[d], 16)

        @block.tensor
        def _(tensor):
            # warm-up: open the HAM clock gate while the loads stream.
            # wu_sb is deliberately uninitialized — the products are never
            # read, only the PE-busy time matters.  Bank 7 is rewritten with
            # start=True by group 7's first matmul ~8 matmuls later, long
            # after the last warmup has drained.
            for _ in range(N_WU):
                nc.tensor.matmul(
                    ps[GP1 - 1][:], wu_sb[:, 0:P], wu_sb[:], start=True, stop=True
                )
            # pass 1: groups 0-7 accumulate in banks 0-7, d outermost so
            # matmuls pace with the streaming loads
            for d in range(D_TILES):
                tensor.wait_ge(d_sems[d], 32)
                for g in range(GP1):
                    m, h = _g_mh(g)
                    mm = nc.tensor.matmul(
                        ps[g][:],
                        ct_sb[d][:, m * P : (m + 1) * P],
                        xt_sb[d][:, h * NF : (h + 1) * NF],
                        start=(d == 0),
                        stop=(d == D_TILES - 1),
                    )
                    if d == D_TILES - 1:
                        mm.then_inc(mm_sem, 1)
            # pass 2: groups 8-15 reuse banks 0-7 once the DVE epilogue has
            # drained the pass-1 group from that bank (P10: concurrent
            # PE-write + DVE-read of one PSUM bank is fatal, so this wait is
            # load-bearing, not just WAR ordering)
            for g in range(GP1, G):
                m, h = _g_mh(g)
                tensor.wait_ge(dve_sem, g - GP1 + 1)
                for d in range(D_TILES):
                    mm = nc.tensor.matmul(
                        ps[g - GP1][:],
                        ct_sb[d][:, m * P : (m + 1) * P],
                        xt_sb[d][:, h * NF : (h + 1) * NF],
                        start=(d == 0),
                        stop=(d == D_TILES - 1),
                    )
                mm.then_inc(mm_sem, 1)

        @block.vector
        def _(vector):
            vector.wait_ge(const_sem, 32)  # ncsq + nxsq present
            for g in range(G - 1):
                m, h = _g_mh(g)
                vector.wait_ge(mm_sem, g + 1)
                nc.vector.scalar_tensor_tensor(
                    ot_sb[:, g * NF : (g + 1) * NF],
                    ps[g % GP1][:],
                    ncsq_sb[:, m : m + 1],
                    nxsq_sb[:, h * NF : (h + 1) * NF],
                    op0=mybir.AluOpType.add,
                    op1=mybir.AluOpType.add,
                ).then_inc(dve_sem, 1)
            m, h = _g_mh(G - 1)
            vector.wait_ge(mm_sem, G)
            for half in range(2):
                nc.vector.scalar_tensor_tensor(
                    ot_sb[
                        :,
                        (G - 1) * NF + half * HNF : (G - 1) * NF + (half + 1) * HNF,
                    ],
                    ps[(G - 1) % GP1][:, half * HNF : (half + 1) * HNF],
                    ncsq_sb[:, m : m + 1],
                    nxsq_sb[:, h * NF + half * HNF : h * NF + (half + 1) * HNF],
                    op0=mybir.AluOpType.add,
                    op1=mybir.AluOpType.add,
                ).then_inc(dve_sem, 1)

    nc.compile()
    return nc


def _get_nc():
    if not hasattr(_cache, "nc"):
        _cache.nc = _build_nc()
    return _cache.nc


def kernel(inputs, centers, _trace=False, _np_dt=None):
    np_dt = _np_dt if _np_dt is not None else _NP_DT
    inputs = np.asarray(inputs, dtype=np.float32)
    centers = np.asarray(centers, dtype=np.float32)

    csq = np.sum(centers.astype(np.float64) ** 2, axis=1)
    xsq = np.sum(inputs.astype(np.float64) ** 2, axis=1)

    ct = np.ascontiguousarray(centers.T).astype(np_dt)
    xt2 = np.ascontiguousarray((2.0 * inputs).T.astype(np_dt))
    ncsq = np.ascontiguousarray((-csq).reshape(M_TILES, P).T.astype(np.float32))

    in_maps = []
    for i in range(N_CORES):
        sl = slice(i * NSH, (i + 1) * NSH)
        in_maps.append(
            {
                "ct": ct,
                "xt": np.ascontiguousarray(xt2[:, sl]),
                "ncsq": ncsq,
                "nxsq": np.ascontiguousarray(
                    np.broadcast_to(-xsq[sl], (P, NSH))
                ).astype(np.float32),
            }
        )

    nc = _get_nc()
    try:
        res = run_bass_kernel_spmd(
            nc, in_maps, core_ids=list(range(N_CORES)), trace=_trace
        )
    except ModuleNotFoundError:
        # NTFF trace glue is absent in some images; rerun without tracing
        res = run_bass_kernel_spmd(
            nc, in_maps, core_ids=list(range(N_CORES)), trace=False
        )
    if _trace:
        kernel.last_results = res
    return np.concatenate([r["out"] for r in res.results], axis=1)


# revision 20
# speedup vs baseline: 1.0409x; 1.0062x over previous
"""Trainium2 Bass kernel for nn_CentersDistance (retrieval_knn).

logits[k, n] = -||centers[k] - inputs[n]||^2
             = 2*(centers @ inputs.T)[k, n] - ||centers[k]||^2 - ||inputs[n]||^2

Strategy (8 NeuronCores, data-parallel over the N=8192 inputs):
  * host: transpose both operands so the contraction dim D lands on the SBUF
    partition axis ([D, K] and [D, N/8] layouts), fold the factor 2 into the
    inputs, and precompute the norm terms exactly in float64.
  * device (per core): a 1024x1024x1024 matmul in bf16 with fp32 PSUM
    accumulation (bf16 streams 1 row/cycle on the PE vs 4 for fp32; the
    measured end-to-end error is absmax/scale 3.3e-4, resid_var 5.4e-9,
    because the exact norm terms dominate the logits).  The epilogue runs on
    the DVE: one scalar_tensor_tensor op adds -||c||^2 (per-partition scalar)
    and -||x||^2 (broadcast row read from a host-precomputed [128, N/8]
    tile), output stored fp32.
  * raw Block/semaphore implementation (not Tile): Tile's ~50 semaphores are
    not the issue (the NRT pre/postamble resets a fixed 51 per engine), but
    Tile adds its own ~6 us drain + clear-semaphores + barrier tail, and its
    scheduler cannot express the exact warmup/pacing we want.
  * the PE is kept continuously busy from ~1 us into the kernel by N_WU
    throwaway matmuls on an (uninitialized) scratch tile so the HAM clock
    gate is fully open (2.4 GHz) when the first real matmul issues; the
    warmup count is sized to bridge until the first ct/xt tile pair lands.
  * loads stream on two HW-DGE queues (Sync: xt, Scalar: ct) with one
    semaphore per d-tile pair: completions of equal-size DMAs are usually in
    issue order, but HBM contention from the other 7 cores can invert them,
    and a single shared counter would then let the PE read a tile that is
    not fully written (observed as a sporadic inf in the output).
  * pass 1 (m-tiles 0-3) runs d outermost so matmuls pace with the streaming
    loads across 8 concurrent PSUM banks; pass 2 (m-tiles 4-7) runs d
    innermost so each output group retires early and its epilogue + store
    overlap the remaining matmuls.

Measured on 8 axon-tunneled trn2 cores: ~45 us NEFF exec (NTFF), of which
~27.6 us is the bf16 PE-stream floor (128 matmuls x 512 rows @ 2.4 GHz) and
~14 us is fixed NRT preamble/postamble (sync barriers, 51-semaphore reset
chains, dma_rearm).

A float32r variant (dt=mybir.dt.float32r, np_dt=np.float32) measures
~56 us / absmax 2.0e-5 — load-bound (8.5 MB vs 4.5 MB of input) but with
near-fp32 precision; kept as a fallback should tighter accuracy ever be
needed.  An fp8e4m3 DoubleRow variant measured ~36 us / absmax 5.2e-3 —
rejected for accuracy-risk reasons.
"""

import threading
from contextlib import ExitStack

import numpy as np
import ml_dtypes

import concourse.mybir as mybir
from concourse import bacc
from concourse.bass_utils import run_bass_kernel_spmd

N_CORES = 8
N, K, D = 8192, 1024, 1024
NSH = N // N_CORES  # per-core slab of inputs
P = 128             # SBUF partitions
NF = 512            # matmul moving free dim (one fp32 PSUM bank)

D_TILES = D // P    # 8 contraction tiles
M_TILES = K // P    # 8 center tiles
H_TILES = NSH // NF # 2 moving-dim tiles

G = M_TILES * H_TILES  # 16 output groups of [128, 512]
GP1 = 8                # groups 0-7 -> pass 1 (m-tiles 0-3), banks 0-7
N_WU = 10              # PE warm-up matmuls

_DT = mybir.dt.bfloat16
_NP_DT = ml_dtypes.bfloat16

_cache = threading.local()


def _g_mh(g):
    return g // H_TILES, g % H_TILES


def _build_nc(dt=_DT):
    nc = bacc.Bacc(
        "TRN2", target_bir_lowering=False, debug=False, num_devices=N_CORES
    )
    ct = nc.dram_tensor("ct", [D, K], dt, kind="ExternalInput").ap()
    xt = nc.dram_tensor("xt", [D, NSH], dt, kind="ExternalInput").ap()
    ncsq = nc.dram_tensor(
        "ncsq", [P, M_TILES], mybir.dt.float32, kind="ExternalInput"
    ).ap()
    nxsq = nc.dram_tensor(
        "nxsq", [P, NSH], mybir.dt.float32, kind="ExternalInput"
    ).ap()
    out = nc.dram_tensor("out", [K, NSH], mybir.dt.float32, kind="ExternalOutput").ap()

    ct_r = ct.rearrange("(t p) k -> t p k", p=P)
    xt_r = xt.rearrange("(t p) n -> t p n", p=P)
    out_r = out.rearrange("(m p) n -> m p n", p=P)

    HNF = NF // 2

    with (
        nc.sbuf_tensor("wu_sb", [P, NF], dt) as wu_sb,
        nc.sbuf_tensor("ncsq_sb", [P, M_TILES], mybir.dt.float32) as ncsq_sb,
        nc.sbuf_tensor("nxsq_sb", [P, NSH], mybir.dt.float32) as nxsq_sb,
        nc.sbuf_tensor("ot_sb", [P, G * NF], mybir.dt.float32) as ot_sb,
        ExitStack() as stack,
        nc.semaphore("const_sem") as const_sem,
        nc.semaphore("mm_sem") as mm_sem,
        nc.semaphore("dve_sem") as dve_sem,
        nc.semaphore("dma_out") as dma_out,
        nc.Block() as block,
    ):
        d_sems = [
            stack.enter_context(nc.semaphore(f"d_sem{i}")) for i in range(D_TILES)
        ]
        ct_sb = [
            stack.enter_context(nc.sbuf_tensor(f"ct_sb{d}", [P, K], dt))
            for d in range(D_TILES)
        ]
        xt_sb = [
            stack.enter_context(nc.sbuf_tensor(f"xt_sb{d}", [P, NSH], dt))
            for d in range(D_TILES)
        ]
        ps = [
            stack.enter_context(nc.psum_tensor(f"ps{b}", [P, NF], mybir.dt.float32))
            for b in range(8)
        ]

        @block.sync
        def _(sync):
            # xt on the Sync HW-DGE queue; ct goes out in parallel on the
            # Scalar engine's queue (block.scalar below) — two rings halve
            # the time to the first d-tile pair and keep the d-loop ahead
            # of the PE throughout
            for d in range(D_TILES):
                sync.dma_start(xt_sb[d][:], xt_r[d]).then_inc(d_sems[d], 16)
            # consts last: only the DVE epilogue (which runs late) needs them
            sync.dma_start(ncsq_sb[:], ncsq).then_inc(const_sem, 16)
            sync.dma_start(nxsq_sb[:], nxsq).then_inc(const_sem, 16)
            for g in range(G - 1):
                m, h = _g_mh(g)
                sync.wait_ge(dve_sem, g + 1)
                sync.dma_start(
                    out_r[m][:, h * NF : (h + 1) * NF],
                    ot_sb[:, g * NF : (g + 1) * NF],
                ).then_inc(dma_out, 16)
            # last group is split in half so its store starts while the DVE
            # is still draining the second half — shorter kernel tail
            m, h = _g_mh(G - 1)
            for half in range(2):
                sync.wait_ge(dve_sem, G + half)
                sync.dma_start(
                    out_r[m][:, h * NF + half * HNF : h * NF + (half + 1) * HNF],
                    ot_sb[
                        :,
                        (G - 1) * NF + half * HNF : (G - 1) * NF + (half + 1) * HNF,
                    ],
                ).then_inc(dma_out, 16)
            sync.wait_ge(dma_out, (G + 1) * 16)

        @block.scalar
        def _(scalar):
            for d in range(D_TILES):
                scalar.dma_start(ct_sb[d][:], ct_r[d]).then_inc(d_sems[d], 16)

        @block.tensor
        def _(tensor):
            # warm-up: open the HAM clock gate while the loads stream.
            # wu_sb is deliberately uninitialized — the products are never
            # read, only the PE-busy time matters.  Bank 7 is rewritten with
            # start=True by group 7's first matmul ~8 matmuls later, long
            # after the last warmup has drained.
            for _ in range(N_WU):
                nc.tensor.matmul(
                    ps[GP1 - 1][:], wu_sb[:, 0:P], wu_sb[:], start=True, stop=True
                )
            # pass 1: groups 0-7 accumulate in banks 0-7, d outermost so
            # matmuls pace with the streaming loads
            for d in range(D_TILES):
                tensor.wait_ge(d_sems[d], 32)
                for g in range(GP1):
                    m, h = _g_mh(g)
                    mm = nc.tensor.matmul(
                        ps[g][:],
                        ct_sb[d][:, m * P : (m + 1) * P],
                        xt_sb[d][:, h * NF : (h + 1) * NF],
                        start=(d == 0),
                        stop=(d == D_TILES - 1),
                    )
                    if d == D_TILES - 1:
                        mm.then_inc(mm_sem, 1)
            # pass 2: groups 8-15 reuse banks 0-7 once the DVE epilogue has
            # drained the pass-1 group from that bank (P10: concurrent
            # PE-write + DVE-read of one PSUM bank is fatal, so this wait is
            # load-bearing, not just WAR ordering)
            for g in range(GP1, G):
                m, h = _g_mh(g)
                if g >= 8:
                    # bank g%8 was last drained by the DVE for group g-8
                    tensor.wait_ge(dve_sem, g - 8 + 1)
                for d in range(D_TILES):
                    mm = nc.tensor.matmul(
                        ps[g % 8][:],
                        ct_sb[d][:, m * P : (m + 1) * P],
                        xt_sb[d][:, h * NF : (h + 1) * NF],
                        start=(d == 0),
                        stop=(d == D_TILES - 1),
                    )
                mm.then_inc(mm_sem, 1)

        @block.vector
        def _(vector):
            vector.wait_ge(const_sem, 32)  # ncsq + nxsq present
            for g in range(G - 1):
                m, h = _g_mh(g)
                vector.wait_ge(mm_sem, g + 1)
                nc.vector.scalar_tensor_tensor(
                    ot_sb[:, g * NF : (g + 1) * NF],
                    ps[g % 8][:],
                    ncsq_sb[:, m : m + 1],
                    nxsq_sb[:, h * NF : (h + 1) * NF],
                    op0=mybir.AluOpType.add,
                    op1=mybir.AluOpType.add,
                ).then_inc(dve_sem, 1)
            m, h = _g_mh(G - 1)
            vector.wait_ge(mm_sem, G)
            for half in range(2):
                nc.vector.scalar_tensor_tensor(
                    ot_sb[
                        :,
                        (G - 1) * NF + half * HNF : (G - 1) * NF + (half + 1) * HNF,
                    ],
                    ps[(G - 1) % 8][:, half * HNF : (half + 1) * HNF],
                    ncsq_sb[:, m : m + 1],
                    nxsq_sb[:, h * NF + half * HNF : h * NF + (half + 1) * HNF],
                    op0=mybir.AluOpType.add,
                    op1=mybir.AluOpType.add,
                ).then_inc(dve_sem, 1)

    nc.compile()
    return nc


def _get_nc():
    if not hasattr(_cache, "nc"):
        _cache.nc = _build_nc()
    return _cache.nc


def kernel(inputs, centers, _trace=False, _np_dt=None):
    np_dt = _np_dt if _np_dt is not None else _NP_DT
    inputs = np.asarray(inputs, dtype=np.float32)
    centers = np.asarray(centers, dtype=np.float32)

    csq = np.sum(centers.astype(np.float64) ** 2, axis=1)
    xsq = np.sum(inputs.astype(np.float64) ** 2, axis=1)

    ct = np.ascontiguousarray(centers.T).astype(np_dt)
    xt2 = np.ascontiguousarray((2.0 * inputs).T.astype(np_dt))
    ncsq = np.ascontiguousarray((-csq).reshape(M_TILES, P).T.astype(np.float32))

    in_maps = []
    for i in range(N_CORES):
        sl = slice(i * NSH, (i + 1) * NSH)
        in_maps.append(
            {
                "ct": ct,
                "xt": np.ascontiguousarray(xt2[:, sl]),
                "ncsq": ncsq,
                "nxsq": np.ascontiguousarray(
                    np.broadcast_to(-xsq[sl], (P, NSH))
                ).astype(np.float32),
            }
        )

    nc = _get_nc()
    try:
        res = run_bass_kernel_spmd(
            nc, in_maps, core_ids=list(range(N_CORES)), trace=_trace
        )
    except ModuleNotFoundError:
        # NTFF trace glue is absent in some images; rerun without tracing
        res = run_bass_kernel_spmd(
            nc, in_maps, core_ids=list(range(N_CORES)), trace=False
        )
    if _trace:
        kernel.last_results = res
    return np.concatenate([r["out"] for r in res.results], axis=1)
